# revision 11
# baseline (speedup 1.0000x reference)
"""Trainium2 Bass kernel for nn_DecomposableAttentionModel.

Math: the reference's softmax is over a size-1 axis, so attn == 1 exactly and

    out[b] = S[b] @ (W2 @ Wo) + N*L*(b2 @ Wo) + bo
    S[b,h] = sum_{n,l} relu(X[b,n,h] + Y[b,l,h])
    X      = gnn(adj, nodes) @ W1[:CG]                  (fp32)
    Y      = prot @ (Wr @ W1[CG:]) + (br @ W1[CG:]) + b1

The [B, N*L, CG+CP] concat tensor is never materialized.

Sharding: data-parallel over batch B=8, one graph per NeuronCore; weights
replicated. No collectives; the host assembles the [8,1] output.

Per-core hot loop: for each n, one fused DVE/ACT instruction computes
relu(Y^T + x_n) and accumulates over the free dim (L=512):
  DVE: tensor_scalar(op0=add(x_n per-partition), op1=max(0), accum_out)
       on bf16 tiles -> 4x perf mode
  ACT: activation(Relu, bias=x_n, accum_out)
HID=192 is split into a 128-partition chunk and a 64-partition chunk; the
64-chunk is packed two-n-per-instruction across the 128 partitions.
"""

import sys

if "/opt/trn_rl_repo" not in sys.path:
    sys.path.insert(0, "/opt/trn_rl_repo")

import numpy as np

import concourse.bacc as bacc
import concourse.bass as bass
import concourse.mybir as mybir
import concourse.tile as tile
from concourse.bass_utils import run_bass_kernel_spmd

B, N, NODE_DIM = 8, 128, 64
L, RES_DIM = 512, 1024
CG, CP = 128, 128
HID = CG + CP // 2  # 192
NUM_GNN_STEPS = 3
NCORES = 8

F32 = mybir.dt.float32
BF16 = mybir.dt.bfloat16
AF = mybir.ActivationFunctionType
ALU = mybir.AluOpType

# fraction of fused-loop units handled by the scalar (ACT) engine; the rest
# run on DVE. Tune from the profile (ACT unit ~2.3x the cost of a DVE unit).
ACT_FRAC = 0.30

_CACHE = {}


def _build():
    nc = bacc.Bacc(
        "TRN2",
        target_bir_lowering=False,
        debug=False,
        num_devices=NCORES,
    )

    def din(name, shape):
        return nc.dram_tensor(name, list(shape), F32, kind="ExternalInput").ap()

    adj = din("adj", (N, N))
    nodesT = din("nodesT", (NODE_DIM, N))
    protT = din("protT", (RES_DIM, L))
    Wn = din("Wn", (NODE_DIM, CG))
    Wg = din("Wg", (CG, CG))
    WrT = din("WrT", (CP, RES_DIM))
    W1t = din("W1t", (CG, HID))
    W1b = din("W1b", (CP, HID))
    W2T = din("W2T", (CG, HID))
    Wo_col = din("Wo_col", (CG, 1))
    bn_row = din("bn_row", (1, CG))
    bg_row = din("bg_row", (1, CG))
    b1_row = din("b1_row", (1, HID))
    br_col = din("br_col", (CP, 1))
    b2_col = din("b2_col", (CG, 1))
    bo11 = din("bo11", (1, 1))
    eye = din("eye", (N, N))
    out_d = nc.dram_tensor("out", [1, 1], F32, kind="ExternalOutput").ap()

    DT = RES_DIM // 128  # 8 k-tiles over the protein feature dim

    with tile.TileContext(nc) as tc:
        with (
            tc.tile_pool(name="const", bufs=1) as cpool,
            tc.tile_pool(name="prot", bufs=1) as ppool,
            tc.tile_pool(name="work", bufs=1) as wpool,
            tc.tile_pool(name="psum", bufs=2, space="PSUM") as psum,
            tc.tile_pool(name="psumY", bufs=1, space="PSUM") as psumY,
        ):
            # ---------------- loads ----------------
            def load(pool, ap, shape, tag, dt=F32):
                t = pool.tile(list(shape), dt, tag=tag)
                nc.sync.dma_start(t[:], ap)
                return t

            s_adj = load(cpool, adj[:, :], (N, N), "adj")
            s_nodesT = load(cpool, nodesT[:, :], (NODE_DIM, N), "nodesT")
            s_Wn = load(cpool, Wn[:, :], (NODE_DIM, CG), "Wn")
            s_Wg = load(cpool, Wg[:, :], (CG, CG), "Wg")
            s_WrT = load(cpool, WrT[:, :], (CP, RES_DIM), "WrT")
            s_W1t = load(cpool, W1t[:, :], (CG, HID), "W1t")
            s_W1b = load(cpool, W1b[:, :], (CP, HID), "W1b")
            s_W2T = load(cpool, W2T[:, :], (CG, HID), "W2T")
            s_Wo = load(cpool, Wo_col[:, :], (CG, 1), "Wo")
            s_bn = load(cpool, bn_row[:, :], (1, CG), "bn")
            s_bg = load(cpool, bg_row[:, :], (1, CG), "bg")
            s_b1 = load(cpool, b1_row[:, :], (1, HID), "b1")
            s_br = load(cpool, br_col[:, :], (CP, 1), "br")
            s_b2 = load(cpool, b2_col[:, :], (CG, 1), "b2")
            s_bo = load(cpool, bo11[:, :], (1, 1), "bo")
            s_eye = load(cpool, eye[:, :], (N, N), "eye")

            s_protT = []
            for d in range(DT):
                s_protT.append(
                    load(ppool, protT[d * 128 : (d + 1) * 128, :], (128, L), f"pt{d}")
                )

            ones1 = cpool.tile([1, 128], F32)
            nc.gpsimd.memset(ones1[:], 1.0)
            ones512 = cpool.tile([1, L], F32)
            nc.gpsimd.memset(ones512[:], 1.0)

            # ---------------- adjacency normalization ----------------
            # At = diag(norm) @ A @ diag(norm), norm = clip(deg,1)^-0.5
            deg = wpool.tile([N, 1], F32)
            nc.vector.tensor_reduce(
                deg[:], s_adj[:], axis=mybir.AxisListType.X, op=ALU.add
            )
            nc.vector.tensor_scalar(deg[:], deg[:], 1.0, None, op0=ALU.max)
            sq = wpool.tile([N, 1], F32)
            nc.scalar.activation(sq[:], deg[:], AF.Sqrt)
            norm = wpool.tile([N, 1], F32)
            nc.vector.reciprocal(norm[:], sq[:])

            rowscaled = wpool.tile([N, N], F32)  # norm_i * A_ij
            nc.vector.tensor_scalar(
                rowscaled[:], s_adj[:], norm[:, 0:1], None, op0=ALU.mult
            )
            ps_rsT = psum.tile([N, N], F32, tag="mm")
            nc.tensor.transpose(ps_rsT[:], rowscaled[:], s_eye[:])
            s_At = wpool.tile([N, N], F32)  # symmetric normalized adjacency
            nc.vector.tensor_scalar(
                s_At[:], ps_rsT[:], norm[:, 0:1], None, op0=ALU.mult
            )

            # ---------------- GNN ----------------
            # h0 = nodes @ Wn + bn   [N, CG]
            ps_h = psum.tile([N, CG], F32, tag="mm")
            nc.tensor.matmul(ps_h[:], s_nodesT[:], s_Wn[:], start=True, stop=False)
            nc.tensor.matmul(ps_h[:], ones1[:], s_bn[:], start=False, stop=True)
            s_h = wpool.tile([N, CG], F32, tag="h")
            nc.scalar.activation(s_h[:], ps_h[:], AF.Copy)

            for step in range(NUM_GNN_STEPS):
                # uT = (At @ h)^T = h^T @ At   (At symmetric)  [CG, N]
                ps_uT = psum.tile([CG, N], F32, tag="uT")
                nc.tensor.matmul(ps_uT[:], s_h[:], s_At[:], start=True, stop=True)
                s_uT = wpool.tile([CG, N], F32, tag="uT_s")
                nc.scalar.activation(s_uT[:], ps_uT[:], AF.Copy)
                # h' = act(u @ Wg + bg)   [N, CG]
                ps_h2 = psum.tile([N, CG], F32, tag="mm")
                nc.tensor.matmul(ps_h2[:], s_uT[:], s_Wg[:], start=True, stop=False)
                nc.tensor.matmul(ps_h2[:], ones1[:], s_bg[:], start=False, stop=True)
                s_h = wpool.tile([N, CG], F32, tag="h")
                nc.scalar.activation(
                    s_h[:],
                    ps_h2[:],
                    AF.Tanh if step == NUM_GNN_STEPS - 1 else AF.Relu,
                )

            # ---------------- X^T = W1t^T @ h3^T   [HID, N] ----------------
            ps_h3T = psum.tile([CG, N], F32, tag="uT")
            nc.tensor.transpose(ps_h3T[:], s_h[:], s_eye[:])
            s_h3T = wpool.tile([CG, N], F32, tag="uT_s")
            nc.scalar.activation(s_h3T[:], ps_h3T[:], AF.Copy)

            ps_XT1 = psum.tile([128, N], F32, tag="mm")
            nc.tensor.matmul(ps_XT1[:], s_W1t[:, 0:128], s_h3T[:], start=True, stop=True)
            s_XT1 = wpool.tile([128, N], F32)
            nc.scalar.activation(s_XT1[:], ps_XT1[:], AF.Copy)

            # chunk-2 biases packed two-per-instruction directly out of PE:
            #   XP2[p<64, j]  = X^T[128+p, j]      (n = j)
            #   XP2[p>=64, j] = X^T[128+p-64, 64+j] (n = 64+j)
            # via two accumulating matmuls with zero-padded stationary tiles.
            w1t2a = wpool.tile([CG, 128], F32)
            w1t2b = wpool.tile([CG, 128], F32)
            nc.gpsimd.memset(w1t2a[:], 0.0)
            nc.gpsimd.memset(w1t2b[:], 0.0)
            nc.vector.tensor_copy(w1t2a[:, 0:64], s_W1t[:, 128:HID])
            nc.vector.tensor_copy(w1t2b[:, 64:128], s_W1t[:, 128:HID])
            ps_XP2 = psum.tile([128, N // 2], F32, tag="mm")
            nc.tensor.matmul(ps_XP2[:], w1t2a[:], s_h3T[:, 0:64], start=True, stop=False)
            nc.tensor.matmul(
                ps_XP2[:], w1t2b[:], s_h3T[:, 64:128], start=False, stop=True
            )
            s_XP2 = wpool.tile([128, N // 2], F32)
            nc.scalar.activation(s_XP2[:], ps_XP2[:], AF.Copy)

            # ---------------- Wc = Wr @ W1b  (-> bf16) ----------------
            s_Wc = []
            for d in range(DT):
                ps_wc = psum.tile([128, HID], F32, tag="mm")
                nc.tensor.matmul(
                    ps_wc[:],
                    s_WrT[:, d * 128 : (d + 1) * 128],
                    s_W1b[:],
                    start=True,
                    stop=True,
                )
                t = wpool.tile([128, HID], BF16, tag=f"wc{d}")
                nc.scalar.activation(t[:], ps_wc[:], AF.Copy)
                s_Wc.append(t)

            # c0 = br @ W1b + b1  (row [1, HID]) folded into Y
            ps_c0 = psum.tile([1, HID], F32, tag="mm")
            nc.tensor.matmul(ps_c0[:], s_br[:], s_W1b[:], start=True, stop=True)
            s_c0 = wpool.tile([1, HID], F32)
            nc.vector.tensor_tensor(s_c0[:], ps_c0[:], s_b1[:], op=ALU.add)

            # chunk-2 stationary tiles with duplicated columns, so the Y2
            # matmul lands already replicated across both partition halves
            s_Wc2r = []
            for d in range(DT):
                t = wpool.tile([128, 128], BF16, tag=f"wc2r{d}")
                nc.vector.tensor_copy(t[:, 0:64], s_Wc[d][:, 128:HID])
                nc.vector.tensor_copy(t[:, 64:128], s_Wc[d][:, 128:HID])
                s_Wc2r.append(t)
            s_c0rep = wpool.tile([1, 128], F32)
            nc.vector.tensor_copy(s_c0rep[:, 0:64], s_c0[:, 128:HID])
            nc.vector.tensor_copy(s_c0rep[:, 64:128], s_c0[:, 128:HID])

            # ---------------- protT -> bf16 ----------------
            s_pbf = []
            for d in range(DT):
                t = ppool.tile([128, L], BF16, tag=f"pbf{d}")
                nc.vector.tensor_copy(t[:], s_protT[d][:])
                s_pbf.append(t)

            # ---------------- Y^T = Wc^T @ protT + c0  [HID, L] ----------------
            ps_Y1 = psumY.tile([128, L], F32, tag="y1")
            ps_Y2 = psumY.tile([128, L], F32, tag="y2")
            for d in range(DT):
                nc.tensor.matmul(
                    ps_Y1[:], s_Wc[d][:, 0:128], s_pbf[d][:], start=(d == 0), stop=False
                )
            nc.tensor.matmul(
                ps_Y1[:], s_c0[:, 0:128], ones512[:], start=False, stop=True
            )
            for d in range(DT):
                nc.tensor.matmul(
                    ps_Y2[:], s_Wc2r[d][:], s_pbf[d][:], start=(d == 0), stop=False
                )
            nc.tensor.matmul(
                ps_Y2[:], s_c0rep[:], ones512[:], start=False, stop=True
            )

            s_Y1 = wpool.tile([128, L], BF16)
            nc.scalar.activation(s_Y1[:], ps_Y1[:], AF.Copy)
            s_Y2r = wpool.tile([128, L], BF16)  # chunk2 duplicated on both halves
            nc.scalar.activation(s_Y2r[:], ps_Y2[:], AF.Copy)

            # ---------------- fused relu-sum loop ----------------
            # Units: chunk1 = 128 n's on (s_Y1, s_XT1); chunk2 = 64 packed j's
            # on (s_Y2r, s_XP2). Each unit u is either:
            #   ACT: one activation(Relu, bias=x_u, accum_out=col)
            #   DVE: relu pass (4x) into a slot of `wide`; one accumulate pass
            #        (4x) per group of G slots produces one column.
            G = 4

            def spread(total, frac):
                k = int(round(total * frac))
                picks = set()
                if k > 0:
                    for i in range(k):
                        picks.add(int(i * total / k))
                return [u in picks for u in range(total)]

            wide = wpool.tile([128, G * L], BF16)
            dump = wpool.tile([128, G * L], BF16)
            scr_a = wpool.tile([128, L], BF16)

            def emit_chunk(ytile, xtile, nunits, sc_a, sc_d):
                on_act = spread(nunits, ACT_FRAC)
                dve_left = sum(1 for f in on_act if not f)
                slot = 0
                ncols_d = 0
                for u in range(nunits):
                    if on_act[u]:
                        nc.scalar.activation(
                            scr_a[:],
                            ytile[:],
                            AF.Relu,
                            bias=xtile[:, u : u + 1],
                            accum_out=sc_a[:, u : u + 1],
                        )
                        continue
                    nc.vector.tensor_scalar(
                        wide[:, slot * L : (slot + 1) * L],
                        ytile[:],
                        xtile[:, u : u + 1],
                        0.0,
                        op0=ALU.add,
                        op1=ALU.max,
                    )
                    slot += 1
                    dve_left -= 1
                    if slot == G or dve_left == 0:
                        w = slot * L
                        nc.vector.tensor_scalar(
                            dump[:, 0:w],
                            wide[:, 0:w],
                            0.0,
                            None,
                            op0=ALU.add,
                            op1=ALU.add,
                            accum_out=sc_d[:, ncols_d : ncols_d + 1],
                        )
                        ncols_d += 1
                        slot = 0
                return ncols_d

            # accumulator columns; zeroed because engines write disjoint cols
            sc_a1 = wpool.tile([128, N], F32)
            sc_d1 = wpool.tile([128, N // G + 2], F32)
            sc_a2 = wpool.tile([128, N // 2], F32)
            sc_d2 = wpool.tile([128, N // 2 // G + 2], F32)
            for t in (sc_a1, sc_d1, sc_a2, sc_d2):
                nc.gpsimd.memset(t[:], 0.0)

            emit_chunk(s_Y1, s_XT1, N, sc_a1, sc_d1)
            emit_chunk(s_Y2r, s_XP2, N // 2, sc_a2, sc_d2)

            # ---------------- reduce S and final dot ----------------
            s_S1 = wpool.tile([128, 1], F32)
            tmp1 = wpool.tile([128, 1], F32)
            nc.vector.tensor_reduce(
                s_S1[:], sc_d1[:], axis=mybir.AxisListType.X, op=ALU.add
            )
            nc.vector.tensor_reduce(
                tmp1[:], sc_a1[:], axis=mybir.AxisListType.X, op=ALU.add
            )
            nc.vector.tensor_tensor(s_S1[:], s_S1[:], tmp1[:], op=ALU.add)

            # S2 packed column (upper half: n<64 sums, lower half: n>=64 sums).
            # No cross-partition fold: dot it against a replicated w2o column.
            r2a = wpool.tile([128, 1], F32)
            r2b = wpool.tile([128, 1], F32)
            nc.vector.tensor_reduce(
                r2a[:], sc_d2[:], axis=mybir.AxisListType.X, op=ALU.add
            )
            nc.vector.tensor_reduce(
                r2b[:], sc_a2[:], axis=mybir.AxisListType.X, op=ALU.add
            )
            nc.vector.tensor_tensor(r2a[:], r2a[:], r2b[:], op=ALU.add)

            # w2o = W2 @ Wo: chunk1 [128,1]; chunk2 replicated to both halves
            ps_w2oa = psum.tile([128, 1], F32, tag="mm")
            nc.tensor.matmul(ps_w2oa[:], s_W2T[:, 0:128], s_Wo[:], start=True, stop=True)
            s_w2oa = wpool.tile([128, 1], F32)
            nc.scalar.activation(s_w2oa[:], ps_w2oa[:], AF.Copy)

            s_W2T2r = wpool.tile([CG, 128], F32)
            nc.vector.tensor_copy(s_W2T2r[:, 0:64], s_W2T[:, 128:HID])
            nc.vector.tensor_copy(s_W2T2r[:, 64:128], s_W2T[:, 128:HID])
            ps_w2ob = psum.tile([128, 1], F32, tag="mm")
            nc.tensor.matmul(ps_w2ob[:], s_W2T2r[:], s_Wo[:], start=True, stop=True)
            s_w2ob = wpool.tile([128, 1], F32)
            nc.scalar.activation(s_w2ob[:], ps_w2ob[:], AF.Copy)

            # b2 * (N*L) for the folded bias term
            s_b2s = wpool.tile([CG, 1], F32)
            nc.vector.tensor_scalar(
                s_b2s[:], s_b2[:], float(N * L), None, op0=ALU.mult
            )

            ps_out = psum.tile([1, 1], F32, tag="mm")
            nc.tensor.matmul(ps_out[:], s_S1[:], s_w2oa[:], start=True, stop=False)
            nc.tensor.matmul(ps_out[:], r2a[:], s_w2ob[:], start=False, stop=False)
            nc.tensor.matmul(ps_out[:], s_b2s[:], s_Wo[:], start=False, stop=True)

            s_out = wpool.tile([1, 1], F32)
            nc.vector.tensor_scalar(
                s_out[:], ps_out[:], s_bo[:, 0:1], None, op0=ALU.add
            )
            nc.sync.dma_start(out_d[:, :], s_out[:])

    nc.compile()
    return nc


def _shard(inputs):
    adj = np.ascontiguousarray(inputs["adj_mats"], np.float32)
    nodes = np.ascontiguousarray(inputs["nodes"], np.float32)
    prot = np.ascontiguousarray(inputs["protein_sequences"], np.float32)
    Wn = np.ascontiguousarray(inputs["Wn"], np.float32)
    Wg = np.ascontiguousarray(inputs["Wg"], np.float32)
    Wr = np.ascontiguousarray(inputs["Wr"], np.float32)
    W1 = np.ascontiguousarray(inputs["W1"], np.float32)
    W2 = np.ascontiguousarray(inputs["W2"], np.float32)
    Wo = np.ascontiguousarray(inputs["Wo"], np.float32)
    bn = np.asarray(inputs["bn"], np.float32)
    bg = np.asarray(inputs["bg"], np.float32)
    br = np.asarray(inputs["br"], np.float32)
    b1 = np.asarray(inputs["b1"], np.float32)
    b2 = np.asarray(inputs["b2"], np.float32)
    bo = np.asarray(inputs["bo"], np.float32)

    shared = {
        "Wn": Wn,
        "Wg": Wg,
        "WrT": np.ascontiguousarray(Wr.T),
        "W1t": np.ascontiguousarray(W1[:CG]),
        "W1b": np.ascontiguousarray(W1[CG:]),
        "W2T": np.ascontiguousarray(W2.T),
        "Wo_col": Wo.reshape(CG, 1),
        "bn_row": bn.reshape(1, CG),
        "bg_row": bg.reshape(1, CG),
        "b1_row": b1.reshape(1, HID),
        "br_col": br.reshape(CP, 1),
        "b2_col": b2.reshape(CG, 1),
        "bo11": bo.reshape(1, 1),
        "eye": np.eye(N, dtype=np.float32),
    }
    in_maps = []
    for b in range(B):
        m = dict(shared)
        m["adj"] = adj[b]
        m["nodesT"] = np.ascontiguousarray(nodes[b].T)
        m["protT"] = np.ascontiguousarray(prot[b].T)
        in_maps.append(m)
    return in_maps


def _ensure_ntff_hook():
    """This container's `antenv` stub lacks axon_hooks; synthesize it from
    trn_boot's ctypes NTFF hook so run_bass_kernel_spmd(trace=True) works."""
    import types

    try:
        from antenv.axon_hooks import get_axon_ntff_profile_hook  # noqa: F401

        return
    except ImportError:
        pass
    try:
        from trn_agent_boot.trn_boot import _ntff_profile_via_ctypes

        hook = _ntff_profile_via_ctypes("/opt/axon/libaxon_pjrt.so")
    except Exception:
        hook = None
    mod = types.ModuleType("antenv.axon_hooks")
    mod._hook = hook
    mod.get_axon_ntff_profile_hook = lambda: mod._hook
    mod.set_axon_ntff_profile_hook = lambda h: setattr(mod, "_hook", h)
    import antenv

    antenv.axon_hooks = mod
    sys.modules["antenv.axon_hooks"] = mod


def _run(inputs, trace=False):
    if "nc" not in _CACHE:
        _CACHE["nc"] = _build()
    nc = _CACHE["nc"]
    if trace:
        _ensure_ntff_hook()
    res = run_bass_kernel_spmd(
        nc, _shard(inputs), core_ids=list(range(NCORES)), trace=trace
    )
    out = np.zeros((B, 1), np.float32)
    for b in range(B):
        out[b, 0] = np.asarray(res.results[b]["out"]).reshape(-1)[0]
    return out, res


def kernel(**inputs) -> np.ndarray:
    out, _ = _run(inputs, trace=False)
    return out


# revision 13
# speedup vs baseline: 1.7581x; 1.7581x over previous
"""Trainium2 Bass kernel for nn_DecomposableAttentionModel.

Math: the reference's softmax is over a size-1 axis, so attn == 1 exactly and

    out[b] = S[b] @ (W2 @ Wo) + N*L*(b2 @ Wo) + bo
    S[b,h] = sum_{n,l} relu(X[b,n,h] + Y[b,l,h])
    X      = gnn(adj, nodes) @ W1[:CG]                  (fp32)
    Y      = prot @ (Wr @ W1[CG:]) + (br @ W1[CG:]) + b1

The [B, N*L, CG+CP] concat tensor is never materialized.

Sharding: data-parallel over batch B=8, one graph per NeuronCore; weights
replicated. No collectives; the host assembles the [8,1] output.

Per-core hot loop: for each n, one fused DVE/ACT instruction computes
relu(Y^T + x_n) and accumulates over the free dim (L=512):
  DVE: tensor_scalar(op0=add(x_n per-partition), op1=max(0), accum_out)
       on bf16 tiles -> 4x perf mode
  ACT: activation(Relu, bias=x_n, accum_out)
HID=192 is split into a 128-partition chunk and a 64-partition chunk; the
64-chunk is packed two-n-per-instruction across the 128 partitions.
"""

import sys

if "/opt/trn_rl_repo" not in sys.path:
    sys.path.insert(0, "/opt/trn_rl_repo")

import numpy as np

import concourse.bacc as bacc
import concourse.bass as bass
import concourse.mybir as mybir
import concourse.tile as tile
from concourse.bass_utils import run_bass_kernel_spmd

B, N, NODE_DIM = 8, 128, 64
L, RES_DIM = 512, 1024
CG, CP = 128, 128
HID = CG + CP // 2  # 192
NUM_GNN_STEPS = 3
NCORES = 8

F32 = mybir.dt.float32
BF16 = mybir.dt.bfloat16
AF = mybir.ActivationFunctionType
ALU = mybir.AluOpType

# fraction of fused-loop units handled by the scalar (ACT) engine; the rest
# run on DVE. Tune from the profile (ACT unit ~2.3x the cost of a DVE unit).
ACT_FRAC = 0.30

_CACHE = {}


def _build():
    nc = bacc.Bacc(
        "TRN2",
        target_bir_lowering=False,
        debug=False,
        num_devices=NCORES,
    )

    def din(name, shape):
        return nc.dram_tensor(name, list(shape), F32, kind="ExternalInput").ap()

    adj = din("adj", (N, N))
    nodesT = din("nodesT", (NODE_DIM, N))
    protT = din("protT", (RES_DIM, L))
    Wn = din("Wn", (NODE_DIM, CG))
    Wg = din("Wg", (CG, CG))
    WrT = din("WrT", (CP, RES_DIM))
    W1t = din("W1t", (CG, HID))
    W1b = din("W1b", (CP, HID))
    W2T = din("W2T", (CG, HID))
    Wo_col = din("Wo_col", (CG, 1))
    bn_row = din("bn_row", (1, CG))
    bg_row = din("bg_row", (1, CG))
    b1_row = din("b1_row", (1, HID))
    br_col = din("br_col", (CP, 1))
    b2_col = din("b2_col", (CG, 1))
    bo11 = din("bo11", (1, 1))
    eye = din("eye", (N, N))
    out_d = nc.dram_tensor("out", [1, 1], F32, kind="ExternalOutput").ap()

    DT = RES_DIM // 128  # 8 k-tiles over the protein feature dim

    with tile.TileContext(nc) as tc:
        with (
            tc.tile_pool(name="const", bufs=1) as cpool,
            tc.tile_pool(name="prot", bufs=1) as ppool,
            tc.tile_pool(name="work", bufs=1) as wpool,
            tc.tile_pool(name="wide", bufs=2) as widepool,
            tc.tile_pool(name="psum", bufs=2, space="PSUM") as psum,
            tc.tile_pool(name="psumY", bufs=1, space="PSUM") as psumY,
        ):
            # ---------------- loads ----------------
            def load(pool, ap, shape, tag, dt=F32):
                t = pool.tile(list(shape), dt, tag=tag)
                nc.sync.dma_start(t[:], ap)
                return t

            s_adj = load(cpool, adj[:, :], (N, N), "adj")
            s_nodesT = load(cpool, nodesT[:, :], (NODE_DIM, N), "nodesT")
            s_Wn = load(cpool, Wn[:, :], (NODE_DIM, CG), "Wn")
            s_Wg = load(cpool, Wg[:, :], (CG, CG), "Wg")
            s_WrT = load(cpool, WrT[:, :], (CP, RES_DIM), "WrT")
            s_W1t = load(cpool, W1t[:, :], (CG, HID), "W1t")
            s_W1b = load(cpool, W1b[:, :], (CP, HID), "W1b")
            s_W2T = load(cpool, W2T[:, :], (CG, HID), "W2T")
            s_Wo = load(cpool, Wo_col[:, :], (CG, 1), "Wo")
            s_bn = load(cpool, bn_row[:, :], (1, CG), "bn")
            s_bg = load(cpool, bg_row[:, :], (1, CG), "bg")
            s_b1 = load(cpool, b1_row[:, :], (1, HID), "b1")
            s_br = load(cpool, br_col[:, :], (CP, 1), "br")
            s_b2 = load(cpool, b2_col[:, :], (CG, 1), "b2")
            s_bo = load(cpool, bo11[:, :], (1, 1), "bo")
            s_eye = load(cpool, eye[:, :], (N, N), "eye")

            s_protT = []
            for d in range(DT):
                s_protT.append(
                    load(ppool, protT[d * 128 : (d + 1) * 128, :], (128, L), f"pt{d}")
                )

            ones1 = cpool.tile([1, 128], F32)
            nc.gpsimd.memset(ones1[:], 1.0)
            ones512 = cpool.tile([1, L], F32)
            nc.gpsimd.memset(ones512[:], 1.0)

            # ---------------- adjacency normalization ----------------
            # At = diag(norm) @ A @ diag(norm), norm = clip(deg,1)^-0.5
            deg = wpool.tile([N, 1], F32)
            nc.vector.tensor_reduce(
                deg[:], s_adj[:], axis=mybir.AxisListType.X, op=ALU.add
            )
            nc.vector.tensor_scalar(deg[:], deg[:], 1.0, None, op0=ALU.max)
            sq = wpool.tile([N, 1], F32)
            nc.scalar.activation(sq[:], deg[:], AF.Sqrt)
            norm = wpool.tile([N, 1], F32)
            nc.vector.reciprocal(norm[:], sq[:])

            rowscaled = wpool.tile([N, N], F32)  # norm_i * A_ij
            nc.vector.tensor_scalar(
                rowscaled[:], s_adj[:], norm[:, 0:1], None, op0=ALU.mult
            )
            ps_rsT = psum.tile([N, N], F32, tag="mm")
            nc.tensor.transpose(ps_rsT[:], rowscaled[:], s_eye[:])
            s_At = wpool.tile([N, N], F32)  # symmetric normalized adjacency
            nc.vector.tensor_scalar(
                s_At[:], ps_rsT[:], norm[:, 0:1], None, op0=ALU.mult
            )

            # ---------------- GNN ----------------
            # h0 = nodes @ Wn + bn   [N, CG]
            ps_h = psum.tile([N, CG], F32, tag="mm")
            nc.tensor.matmul(ps_h[:], s_nodesT[:], s_Wn[:], start=True, stop=False)
            nc.tensor.matmul(ps_h[:], ones1[:], s_bn[:], start=False, stop=True)
            s_h = wpool.tile([N, CG], F32, tag="h")
            nc.scalar.activation(s_h[:], ps_h[:], AF.Copy)

            for step in range(NUM_GNN_STEPS):
                # uT = (At @ h)^T = h^T @ At   (At symmetric)  [CG, N]
                ps_uT = psum.tile([CG, N], F32, tag="uT")
                nc.tensor.matmul(ps_uT[:], s_h[:], s_At[:], start=True, stop=True)
                s_uT = wpool.tile([CG, N], F32, tag="uT_s")
                nc.scalar.activation(s_uT[:], ps_uT[:], AF.Copy)
                # h' = act(u @ Wg + bg)   [N, CG]
                ps_h2 = psum.tile([N, CG], F32, tag="mm")
                nc.tensor.matmul(ps_h2[:], s_uT[:], s_Wg[:], start=True, stop=False)
                nc.tensor.matmul(ps_h2[:], ones1[:], s_bg[:], start=False, stop=True)
                s_h = wpool.tile([N, CG], F32, tag="h")
                nc.scalar.activation(
                    s_h[:],
                    ps_h2[:],
                    AF.Tanh if step == NUM_GNN_STEPS - 1 else AF.Relu,
                )

            # ---------------- X^T = W1t^T @ h3^T   [HID, N] ----------------
            ps_h3T = psum.tile([CG, N], F32, tag="uT")
            nc.tensor.transpose(ps_h3T[:], s_h[:], s_eye[:])
            s_h3T = wpool.tile([CG, N], F32, tag="uT_s")
            nc.scalar.activation(s_h3T[:], ps_h3T[:], AF.Copy)

            ps_XT1 = psum.tile([128, N], F32, tag="mm")
            nc.tensor.matmul(ps_XT1[:], s_W1t[:, 0:128], s_h3T[:], start=True, stop=True)
            s_XT1 = wpool.tile([128, N], F32)
            nc.scalar.activation(s_XT1[:], ps_XT1[:], AF.Copy)

            # chunk-2 biases packed two-per-instruction directly out of PE:
            #   XP2[p<64, j]  = X^T[128+p, j]      (n = j)
            #   XP2[p>=64, j] = X^T[128+p-64, 64+j] (n = 64+j)
            # via two accumulating matmuls with zero-padded stationary tiles.
            w1t2a = wpool.tile([CG, 128], F32)
            w1t2b = wpool.tile([CG, 128], F32)
            nc.gpsimd.memset(w1t2a[:], 0.0)
            nc.gpsimd.memset(w1t2b[:], 0.0)
            nc.vector.tensor_copy(w1t2a[:, 0:64], s_W1t[:, 128:HID])
            nc.vector.tensor_copy(w1t2b[:, 64:128], s_W1t[:, 128:HID])
            ps_XP2 = psum.tile([128, N // 2], F32, tag="mm")
            nc.tensor.matmul(ps_XP2[:], w1t2a[:], s_h3T[:, 0:64], start=True, stop=False)
            nc.tensor.matmul(
                ps_XP2[:], w1t2b[:], s_h3T[:, 64:128], start=False, stop=True
            )
            s_XP2 = wpool.tile([128, N // 2], F32)
            nc.scalar.activation(s_XP2[:], ps_XP2[:], AF.Copy)

            # ---------------- Wc = Wr @ W1b  (-> bf16) ----------------
            s_Wc = []
            for d in range(DT):
                ps_wc = psum.tile([128, HID], F32, tag="mm")
                nc.tensor.matmul(
                    ps_wc[:],
                    s_WrT[:, d * 128 : (d + 1) * 128],
                    s_W1b[:],
                    start=True,
                    stop=True,
                )
                t = wpool.tile([128, HID], BF16, tag=f"wc{d}")
                nc.scalar.activation(t[:], ps_wc[:], AF.Copy)
                s_Wc.append(t)

            # c0 = br @ W1b + b1  (row [1, HID]) folded into Y
            ps_c0 = psum.tile([1, HID], F32, tag="mm")
            nc.tensor.matmul(ps_c0[:], s_br[:], s_W1b[:], start=True, stop=True)
            s_c0 = wpool.tile([1, HID], F32)
            nc.vector.tensor_tensor(s_c0[:], ps_c0[:], s_b1[:], op=ALU.add)

            # chunk-2 stationary tiles with duplicated columns, so the Y2
            # matmul lands already replicated across both partition halves
            s_Wc2r = []
            for d in range(DT):
                t = wpool.tile([128, 128], BF16, tag=f"wc2r{d}")
                nc.vector.tensor_copy(t[:, 0:64], s_Wc[d][:, 128:HID])
                nc.vector.tensor_copy(t[:, 64:128], s_Wc[d][:, 128:HID])
                s_Wc2r.append(t)
            s_c0rep = wpool.tile([1, 128], F32)
            nc.vector.tensor_copy(s_c0rep[:, 0:64], s_c0[:, 128:HID])
            nc.vector.tensor_copy(s_c0rep[:, 64:128], s_c0[:, 128:HID])

            # ---------------- protT -> bf16 ----------------
            s_pbf = []
            for d in range(DT):
                t = ppool.tile([128, L], BF16, tag=f"pbf{d}")
                nc.vector.tensor_copy(t[:], s_protT[d][:])
                s_pbf.append(t)

            # ---------------- Y^T = Wc^T @ protT + c0  [HID, L] ----------------
            ps_Y1 = psumY.tile([128, L], F32, tag="y1")
            ps_Y2 = psumY.tile([128, L], F32, tag="y2")
            for d in range(DT):
                nc.tensor.matmul(
                    ps_Y1[:], s_Wc[d][:, 0:128], s_pbf[d][:], start=(d == 0), stop=False
                )
            nc.tensor.matmul(
                ps_Y1[:], s_c0[:, 0:128], ones512[:], start=False, stop=True
            )
            for d in range(DT):
                nc.tensor.matmul(
                    ps_Y2[:], s_Wc2r[d][:], s_pbf[d][:], start=(d == 0), stop=False
                )
            nc.tensor.matmul(
                ps_Y2[:], s_c0rep[:], ones512[:], start=False, stop=True
            )

            s_Y1 = wpool.tile([128, L], BF16)
            nc.scalar.activation(s_Y1[:], ps_Y1[:], AF.Copy)
            s_Y2r = wpool.tile([128, L], BF16)  # chunk2 duplicated on both halves
            nc.scalar.activation(s_Y2r[:], ps_Y2[:], AF.Copy)

            # ---------------- w2o = W2 @ Wo (bf16 columns for PE reduce) ----
            # chunk1: plain [128,1]; chunk2: replicated to both partition halves
            ps_w2oa = psum.tile([128, 1], F32, tag="mm")
            nc.tensor.matmul(ps_w2oa[:], s_W2T[:, 0:128], s_Wo[:], start=True, stop=True)
            w2o1c = wpool.tile([128, 1], BF16)
            nc.scalar.activation(w2o1c[:], ps_w2oa[:], AF.Copy)

            s_W2T2r = wpool.tile([CG, 128], F32)
            nc.vector.tensor_copy(s_W2T2r[:, 0:64], s_W2T[:, 128:HID])
            nc.vector.tensor_copy(s_W2T2r[:, 64:128], s_W2T[:, 128:HID])
            ps_w2ob = psum.tile([128, 1], F32, tag="mm")
            nc.tensor.matmul(ps_w2ob[:], s_W2T2r[:], s_Wo[:], start=True, stop=True)
            w2o2c = wpool.tile([128, 1], BF16)
            nc.scalar.activation(w2o2c[:], ps_w2ob[:], AF.Copy)

            # ---------------- fused relu-sum loop ----------------
            # Units: chunk1 = 128 n's on (s_Y1, s_XT1, w2o1c); chunk2 = 64
            # packed j's on (s_Y2r, s_XP2, w2o2c). Every unit: one relu pass
            # (DVE tensor_scalar 2x, or ACT activation) into a slot of a wide
            # bf16 buffer; PE then folds each slot into ps_red[1, 512] as
            # w2o^T @ slice, accumulating the w2o-weighted relu sums for all
            # units directly in PSUM.
            G = 4

            def spread(total, frac):
                k = int(round(total * frac))
                picks = set()
                for i in range(k):
                    picks.add(int(i * total / k))
                return [u in picks for u in range(total)]

            ps_red = psumY.tile([1, L], F32, tag="red")
            chunks = [(s_Y1, s_XT1, w2o1c, N), (s_Y2r, s_XP2, w2o2c, N // 2)]
            assigns = [spread(nu, ACT_FRAC) for (_, _, _, nu) in chunks]
            total_mms = sum(nu for (_, _, _, nu) in chunks)
            mm_idx = [0]

            def pe_flush(widetile, nslots, w2oc):
                for s in range(nslots):
                    nc.tensor.matmul(
                        ps_red[:],
                        w2oc[:],
                        widetile[:, s * L : (s + 1) * L],
                        start=(mm_idx[0] == 0),
                        stop=(mm_idx[0] == total_mms - 1),
                    )
                    mm_idx[0] += 1

            for (ytile, xtile, w2oc, nunits), on_act in zip(chunks, assigns):
                n_act_left = sum(on_act)
                n_dve_left = nunits - n_act_left
                wd = widepool.tile([128, G * L], BF16, tag="wd")
                wa = widepool.tile([128, G * L], BF16, tag="wa")
                ds = asl = 0
                for u in range(nunits):
                    if on_act[u]:
                        nc.scalar.activation(
                            wa[:, asl * L : (asl + 1) * L],
                            ytile[:],
                            AF.Relu,
                            bias=xtile[:, u : u + 1],
                        )
                        asl += 1
                        n_act_left -= 1
                        if asl == G or n_act_left == 0:
                            pe_flush(wa, asl, w2oc)
                            wa = widepool.tile([128, G * L], BF16, tag="wa")
                            asl = 0
                    else:
                        nc.vector.tensor_scalar(
                            wd[:, ds * L : (ds + 1) * L],
                            ytile[:],
                            xtile[:, u : u + 1],
                            0.0,
                            op0=ALU.add,
                            op1=ALU.max,
                        )
                        ds += 1
                        n_dve_left -= 1
                        if ds == G or n_dve_left == 0:
                            pe_flush(wd, ds, w2oc)
                            wd = widepool.tile([128, G * L], BF16, tag="wd")
                            ds = 0

            # ---------------- final scalar ----------------
            red_row = wpool.tile([1, L], F32)
            nc.scalar.activation(red_row[:], ps_red[:], AF.Copy)
            red_sum = wpool.tile([1, 1], F32)
            nc.vector.tensor_reduce(
                red_sum[:], red_row[:], axis=mybir.AxisListType.X, op=ALU.add
            )

            # b2 * (N*L) folded bias term
            s_b2s = wpool.tile([CG, 1], F32)
            nc.vector.tensor_scalar(
                s_b2s[:], s_b2[:], float(N * L), None, op0=ALU.mult
            )
            ps_out = psum.tile([1, 1], F32, tag="mm")
            nc.tensor.matmul(ps_out[:], s_b2s[:], s_Wo[:], start=True, stop=True)
            bterm = wpool.tile([1, 1], F32)
            nc.vector.tensor_scalar(
                bterm[:], ps_out[:], s_bo[:, 0:1], None, op0=ALU.add
            )

            s_out = wpool.tile([1, 1], F32)
            nc.vector.tensor_tensor(s_out[:], red_sum[:], bterm[:], op=ALU.add)
            nc.sync.dma_start(out_d[:, :], s_out[:])

    nc.compile()
    return nc


def _shard(inputs):
    adj = np.ascontiguousarray(inputs["adj_mats"], np.float32)
    nodes = np.ascontiguousarray(inputs["nodes"], np.float32)
    prot = np.ascontiguousarray(inputs["protein_sequences"], np.float32)
    Wn = np.ascontiguousarray(inputs["Wn"], np.float32)
    Wg = np.ascontiguousarray(inputs["Wg"], np.float32)
    Wr = np.ascontiguousarray(inputs["Wr"], np.float32)
    W1 = np.ascontiguousarray(inputs["W1"], np.float32)
    W2 = np.ascontiguousarray(inputs["W2"], np.float32)
    Wo = np.ascontiguousarray(inputs["Wo"], np.float32)
    bn = np.asarray(inputs["bn"], np.float32)
    bg = np.asarray(inputs["bg"], np.float32)
    br = np.asarray(inputs["br"], np.float32)
    b1 = np.asarray(inputs["b1"], np.float32)
    b2 = np.asarray(inputs["b2"], np.float32)
    bo = np.asarray(inputs["bo"], np.float32)

    shared = {
        "Wn": Wn,
        "Wg": Wg,
        "WrT": np.ascontiguousarray(Wr.T),
        "W1t": np.ascontiguousarray(W1[:CG]),
        "W1b": np.ascontiguousarray(W1[CG:]),
        "W2T": np.ascontiguousarray(W2.T),
        "Wo_col": Wo.reshape(CG, 1),
        "bn_row": bn.reshape(1, CG),
        "bg_row": bg.reshape(1, CG),
        "b1_row": b1.reshape(1, HID),
        "br_col": br.reshape(CP, 1),
        "b2_col": b2.reshape(CG, 1),
        "bo11": bo.reshape(1, 1),
        "eye": np.eye(N, dtype=np.float32),
    }
    in_maps = []
    for b in range(B):
        m = dict(shared)
        m["adj"] = adj[b]
        m["nodesT"] = np.ascontiguousarray(nodes[b].T)
        m["protT"] = np.ascontiguousarray(prot[b].T)
        in_maps.append(m)
    return in_maps


def _ensure_ntff_hook():
    """This container's `antenv` stub lacks axon_hooks; synthesize it from
    trn_boot's ctypes NTFF hook so run_bass_kernel_spmd(trace=True) works."""
    import types

    try:
        from antenv.axon_hooks import get_axon_ntff_profile_hook  # noqa: F401

        return
    except ImportError:
        pass
    try:
        from trn_agent_boot.trn_boot import _ntff_profile_via_ctypes

        hook = _ntff_profile_via_ctypes("/opt/axon/libaxon_pjrt.so")
    except Exception:
        hook = None
    mod = types.ModuleType("antenv.axon_hooks")
    mod._hook = hook
    mod.get_axon_ntff_profile_hook = lambda: mod._hook
    mod.set_axon_ntff_profile_hook = lambda h: setattr(mod, "_hook", h)
    import antenv

    antenv.axon_hooks = mod
    sys.modules["antenv.axon_hooks"] = mod


def _run(inputs, trace=False):
    if "nc" not in _CACHE:
        _CACHE["nc"] = _build()
    nc = _CACHE["nc"]
    if trace:
        _ensure_ntff_hook()
    res = run_bass_kernel_spmd(
        nc, _shard(inputs), core_ids=list(range(NCORES)), trace=trace
    )
    out = np.zeros((B, 1), np.float32)
    for b in range(B):
        out[b, 0] = np.asarray(res.results[b]["out"]).reshape(-1)[0]
    return out, res


def kernel(**inputs) -> np.ndarray:
    out, _ = _run(inputs, trace=False)
    return out


# revision 15
# speedup vs baseline: 1.8379x; 1.0454x over previous
"""Trainium2 Bass kernel for nn_DecomposableAttentionModel.

Math: the reference's softmax is over a size-1 axis, so attn == 1 exactly and

    out[b] = S[b] @ (W2 @ Wo) + N*L*(b2 @ Wo) + bo
    S[b,h] = sum_{n,l} relu(X[b,n,h] + Y[b,l,h])
    X      = gnn(adj, nodes) @ W1[:CG]
    Y      = prot @ (Wr @ W1[CG:]) + (br @ W1[CG:]) + b1

The [B, N*L, CG+CP] concat tensor is never materialized. Since
w*relu(z) = sign(w)*relu(|w|*z), |w2o| is folded into X and Y at PSUM
evacuation (free per-partition scale on the ACT copy) and the PE reduce
uses exact sign(w2o) stationary vectors.

Sharding: data-parallel over batch B=8, one graph per NeuronCore; weights
replicated. No collectives; the host assembles the [8,1] output. All
weights/biases ship as one packed [128, WCOLS] blob -> a single DMA.

Per-core hot loop (192 units: 128 chunk1 n's + 64 packed chunk2 pairs):
every unit is one relu pass (DVE tensor_scalar 2x | ACT activation) of
Y'[128,512]+x'_u into a wide bf16 buffer; PE folds each slot into
ps_red[1,512] via sign(w2o)^T @ slice matmuls accumulated in PSUM.
"""

import sys

if "/opt/trn_rl_repo" not in sys.path:
    sys.path.insert(0, "/opt/trn_rl_repo")

import numpy as np

import concourse.bacc as bacc
import concourse.bass as bass
import concourse.mybir as mybir
import concourse.tile as tile
from concourse.bass_utils import run_bass_kernel_spmd

B, N, NODE_DIM = 8, 128, 64
L, RES_DIM = 512, 1024
CG, CP = 128, 128
HID = CG + CP // 2  # 192
NUM_GNN_STEPS = 3
NCORES = 8

F32 = mybir.dt.float32
BF16 = mybir.dt.bfloat16
AF = mybir.ActivationFunctionType
ALU = mybir.AluOpType

# fraction of fused-loop units on the scalar (ACT) engine; rest on DVE.
ACT_FRAC = 0.30
G = 4  # wide-buffer slots per flush

# packed weight-blob column layout: name -> (col_offset, cols)
_WB = {}
_c = 0
for _name, _w in [
    ("adj", N), ("eye", N), ("Wg", CG), ("WrT", RES_DIM), ("W1t", HID),
    ("W1b", HID), ("W2T", HID), ("Wn", N), ("nodesT", N), ("cols3", 3),
    ("rowvec", 449),
]:
    _WB[_name] = (_c, _w)
    _c += _w
WCOLS = _c

_CACHE = {}


def _build():
    nc = bacc.Bacc(
        "TRN2",
        target_bir_lowering=False,
        debug=False,
        num_devices=NCORES,
    )

    wblob = nc.dram_tensor("wblob", [128, WCOLS], F32, kind="ExternalInput").ap()
    protT = nc.dram_tensor("protT", [RES_DIM, L], F32, kind="ExternalInput").ap()
    out_d = nc.dram_tensor("out", [1, 1], F32, kind="ExternalOutput").ap()

    DT = RES_DIM // 128  # 8 k-tiles over the protein feature dim

    with tile.TileContext(nc) as tc:
        with (
            tc.tile_pool(name="const", bufs=1) as cpool,
            tc.tile_pool(name="work", bufs=1) as wpool,
            tc.tile_pool(name="wide", bufs=2) as widepool,
            tc.tile_pool(name="psum", bufs=2, space="PSUM") as psum,
            tc.tile_pool(name="psumY", bufs=1, space="PSUM") as psumY,
        ):
            # ---------------- loads (3 DMAs total) ----------------
            wb = cpool.tile([128, WCOLS], F32, tag="wblob")
            nc.sync.dma_start(wb[:], wblob[:, :])

            s_protall = cpool.tile([128, DT * L], F32, tag="prot")
            pt3 = protT.rearrange("(d p) l -> p d l", p=128)
            half = DT // 2
            nc.sync.dma_start(
                s_protall[:, 0 : half * L].rearrange("p (d l) -> p d l", d=half),
                pt3[:, 0:half, :],
            )
            nc.sync.dma_start(
                s_protall[:, half * L : DT * L].rearrange(
                    "p (d l) -> p d l", d=half
                ),
                pt3[:, half:DT, :],
            )

            def wbs(name, rows=slice(0, 128), coff=0, cols=None):
                c0, cw = _WB[name]
                return wb[rows, c0 + coff : c0 + coff + (cols or cw)]

            s_adj = wbs("adj")
            s_eye = wbs("eye")
            s_Wg = wbs("Wg")
            s_WrT = wbs("WrT")
            s_W1t = wbs("W1t")
            s_W1b = wbs("W1b")
            s_W2T = wbs("W2T")
            s_Wn = wbs("Wn", rows=slice(0, 64))
            s_nodesT = wbs("nodesT", rows=slice(0, 64))
            s_Wo = wbs("cols3", coff=0, cols=1)
            s_br = wbs("cols3", coff=1, cols=1)
            s_b2 = wbs("cols3", coff=2, cols=1)
            s_bn = wbs("rowvec", rows=slice(0, 1), coff=0, cols=CG)
            s_bg = wbs("rowvec", rows=slice(0, 1), coff=128, cols=CG)
            s_b1 = wbs("rowvec", rows=slice(0, 1), coff=256, cols=HID)
            s_bo = wbs("rowvec", rows=slice(0, 1), coff=448, cols=1)

            ones1 = cpool.tile([1, 128], F32)
            nc.gpsimd.memset(ones1[:], 1.0)
            ones512 = cpool.tile([1, L], F32)
            nc.gpsimd.memset(ones512[:], 1.0)

            # ---------------- w2o = W2 @ Wo; |w2o| scales + sign columns ----
            ps_w2oa = psum.tile([128, 1], F32, tag="mm")
            nc.tensor.matmul(ps_w2oa[:], s_W2T[:, 0:128], s_Wo[:], start=True, stop=True)
            absw1 = wpool.tile([128, 1], F32)
            nc.scalar.activation(absw1[:], ps_w2oa[:], AF.Abs)
            sgn1 = wpool.tile([128, 1], BF16)
            nc.scalar.activation(sgn1[:], ps_w2oa[:], AF.Sign)

            s_W2T2r = wpool.tile([CG, 128], F32)
            nc.vector.tensor_copy(s_W2T2r[:, 0:64], s_W2T[:, 128:HID])
            nc.vector.tensor_copy(s_W2T2r[:, 64:128], s_W2T[:, 128:HID])
            ps_w2ob = psum.tile([128, 1], F32, tag="mm")
            nc.tensor.matmul(ps_w2ob[:], s_W2T2r[:], s_Wo[:], start=True, stop=True)
            absw2 = wpool.tile([128, 1], F32)
            nc.scalar.activation(absw2[:], ps_w2ob[:], AF.Abs)
            sgn2 = wpool.tile([128, 1], BF16)
            nc.scalar.activation(sgn2[:], ps_w2ob[:], AF.Sign)

            # ---------------- adjacency normalization ----------------
            # At = diag(norm) @ A @ diag(norm), norm = clip(deg,1)^-0.5
            deg = wpool.tile([N, 1], F32)
            nc.vector.tensor_reduce(
                deg[:], s_adj[:], axis=mybir.AxisListType.X, op=ALU.add
            )
            nc.vector.tensor_scalar(deg[:], deg[:], 1.0, None, op0=ALU.max)
            sq = wpool.tile([N, 1], F32)
            nc.scalar.activation(sq[:], deg[:], AF.Sqrt)
            norm = wpool.tile([N, 1], F32)
            nc.vector.reciprocal(norm[:], sq[:])

            rowscaled = wpool.tile([N, N], F32)  # norm_i * A_ij
            nc.vector.tensor_scalar(
                rowscaled[:], s_adj[:], norm[:, 0:1], None, op0=ALU.mult
            )
            ps_rsT = psum.tile([N, N], F32, tag="mm")
            nc.tensor.transpose(ps_rsT[:], rowscaled[:], s_eye[:])
            s_At = wpool.tile([N, N], F32)  # symmetric normalized adjacency
            nc.vector.tensor_scalar(
                s_At[:], ps_rsT[:], norm[:, 0:1], None, op0=ALU.mult
            )

            # ---------------- GNN ----------------
            # h0 = nodes @ Wn + bn   [N, CG]
            ps_h = psum.tile([N, CG], F32, tag="mm")
            nc.tensor.matmul(ps_h[:], s_nodesT[:], s_Wn[:], start=True, stop=False)
            nc.tensor.matmul(ps_h[:], ones1[:], s_bn[:], start=False, stop=True)
            s_h = wpool.tile([N, CG], F32, tag="h")
            nc.scalar.activation(s_h[:], ps_h[:], AF.Copy)

            for step in range(NUM_GNN_STEPS):
                # uT = (At @ h)^T = h^T @ At   (At symmetric)  [CG, N]
                ps_uT = psum.tile([CG, N], F32, tag="uT")
                nc.tensor.matmul(ps_uT[:], s_h[:], s_At[:], start=True, stop=True)
                s_uT = wpool.tile([CG, N], F32, tag="uT_s")
                nc.scalar.activation(s_uT[:], ps_uT[:], AF.Copy)
                # h' = act(u @ Wg + bg)   [N, CG]
                ps_h2 = psum.tile([N, CG], F32, tag="mm")
                nc.tensor.matmul(ps_h2[:], s_uT[:], s_Wg[:], start=True, stop=False)
                nc.tensor.matmul(ps_h2[:], ones1[:], s_bg[:], start=False, stop=True)
                s_h = wpool.tile([N, CG], F32, tag="h")
                nc.scalar.activation(
                    s_h[:],
                    ps_h2[:],
                    AF.Tanh if step == NUM_GNN_STEPS - 1 else AF.Relu,
                )

            # ---------------- X^T = W1t^T @ h3^T, scaled by |w2o| ----------
            ps_h3T = psum.tile([CG, N], F32, tag="uT")
            nc.tensor.transpose(ps_h3T[:], s_h[:], s_eye[:])
            s_h3T = wpool.tile([CG, N], F32, tag="uT_s")
            nc.scalar.activation(s_h3T[:], ps_h3T[:], AF.Copy)

            ps_XT1 = psum.tile([128, N], F32, tag="mm")
            nc.tensor.matmul(ps_XT1[:], s_W1t[:, 0:128], s_h3T[:], start=True, stop=True)
            s_XT1 = wpool.tile([128, N], F32)
            nc.scalar.activation(s_XT1[:], ps_XT1[:], AF.Copy, scale=absw1[:, 0:1])

            # chunk-2 biases packed two-per-instruction directly out of PE:
            #   XP2[p<64, j] = X^T[128+p, j]; XP2[p>=64, j] = X^T[128+p-64, 64+j]
            w1t2a = wpool.tile([CG, 128], F32)
            w1t2b = wpool.tile([CG, 128], F32)
            nc.gpsimd.memset(w1t2a[:], 0.0)
            nc.gpsimd.memset(w1t2b[:], 0.0)
            nc.vector.tensor_copy(w1t2a[:, 0:64], s_W1t[:, 128:HID])
            nc.vector.tensor_copy(w1t2b[:, 64:128], s_W1t[:, 128:HID])
            ps_XP2 = psum.tile([128, N // 2], F32, tag="mm")
            nc.tensor.matmul(ps_XP2[:], w1t2a[:], s_h3T[:, 0:64], start=True, stop=False)
            nc.tensor.matmul(
                ps_XP2[:], w1t2b[:], s_h3T[:, 64:128], start=False, stop=True
            )
            s_XP2 = wpool.tile([128, N // 2], F32)
            nc.scalar.activation(s_XP2[:], ps_XP2[:], AF.Copy, scale=absw2[:, 0:1])

            # ---------------- Wc = Wr @ W1b  (-> bf16) ----------------
            s_Wc = []
            for d in range(DT):
                ps_wc = psum.tile([128, HID], F32, tag="mm")
                nc.tensor.matmul(
                    ps_wc[:],
                    s_WrT[:, d * 128 : (d + 1) * 128],
                    s_W1b[:],
                    start=True,
                    stop=True,
                )
                t = wpool.tile([128, HID], BF16, tag=f"wc{d}")
                nc.scalar.activation(t[:], ps_wc[:], AF.Copy)
                s_Wc.append(t)

            # c0 = br @ W1b + b1  (row [1, HID]) folded into Y
            ps_c0 = psum.tile([1, HID], F32, tag="mm")
            nc.tensor.matmul(ps_c0[:], s_br[:], s_W1b[:], start=True, stop=True)
            s_c0 = wpool.tile([1, HID], F32)
            nc.vector.tensor_tensor(s_c0[:], ps_c0[:], s_b1[:], op=ALU.add)

            # chunk-2 stationary tiles with duplicated columns, so the Y2
            # matmul lands already replicated across both partition halves
            s_Wc2r = []
            for d in range(DT):
                t = wpool.tile([128, 128], BF16, tag=f"wc2r{d}")
                nc.vector.tensor_copy(t[:, 0:64], s_Wc[d][:, 128:HID])
                nc.vector.tensor_copy(t[:, 64:128], s_Wc[d][:, 128:HID])
                s_Wc2r.append(t)
            s_c0rep = wpool.tile([1, 128], F32)
            nc.vector.tensor_copy(s_c0rep[:, 0:64], s_c0[:, 128:HID])
            nc.vector.tensor_copy(s_c0rep[:, 64:128], s_c0[:, 128:HID])

            # ---------------- protT -> bf16 ----------------
            s_pbfall = cpool.tile([128, DT * L], BF16, tag="pbf")
            for d in range(DT):
                nc.vector.tensor_copy(
                    s_pbfall[:, d * L : (d + 1) * L],
                    s_protall[:, d * L : (d + 1) * L],
                )

            def pbf(d):
                return s_pbfall[:, d * L : (d + 1) * L]

            # ------- Y^T = Wc^T @ protT + c0, scaled by |w2o|  [HID, L] -----
            ps_Y1 = psumY.tile([128, L], F32, tag="y1")
            ps_Y2 = psumY.tile([128, L], F32, tag="y2")
            for d in range(DT):
                nc.tensor.matmul(
                    ps_Y1[:], s_Wc[d][:, 0:128], pbf(d), start=(d == 0), stop=False
                )
            nc.tensor.matmul(
                ps_Y1[:], s_c0[:, 0:128], ones512[:], start=False, stop=True
            )
            for d in range(DT):
                nc.tensor.matmul(
                    ps_Y2[:], s_Wc2r[d][:], pbf(d), start=(d == 0), stop=False
                )
            nc.tensor.matmul(
                ps_Y2[:], s_c0rep[:], ones512[:], start=False, stop=True
            )

            s_Y1 = wpool.tile([128, L], BF16)
            nc.scalar.activation(s_Y1[:], ps_Y1[:], AF.Copy, scale=absw1[:, 0:1])
            s_Y2r = wpool.tile([128, L], BF16)
            nc.scalar.activation(s_Y2r[:], ps_Y2[:], AF.Copy, scale=absw2[:, 0:1])

            # ---------------- fused relu-sum loop ----------------
            def spread(total, frac):
                k = int(round(total * frac))
                picks = set()
                for i in range(k):
                    picks.add(int(i * total / k))
                return [u in picks for u in range(total)]

            ps_red = psumY.tile([1, L], F32, tag="red")
            chunks = [(s_Y1, s_XT1, sgn1, N), (s_Y2r, s_XP2, sgn2, N // 2)]
            assigns = [spread(nu, ACT_FRAC) for (_, _, _, nu) in chunks]
            total_mms = sum(nu for (_, _, _, nu) in chunks)
            mm_idx = [0]

            def pe_flush(widetile, nslots, sgnc):
                for s in range(nslots):
                    nc.tensor.matmul(
                        ps_red[:],
                        sgnc[:],
                        widetile[:, s * L : (s + 1) * L],
                        start=(mm_idx[0] == 0),
                        stop=(mm_idx[0] == total_mms - 1),
                    )
                    mm_idx[0] += 1

            for (ytile, xtile, sgnc, nunits), on_act in zip(chunks, assigns):
                n_act_left = sum(on_act)
                n_dve_left = nunits - n_act_left
                wd = widepool.tile([128, G * L], BF16, tag="wd")
                wa = widepool.tile([128, G * L], BF16, tag="wa")
                ds = asl = 0
                for u in range(nunits):
                    if on_act[u]:
                        nc.scalar.activation(
                            wa[:, asl * L : (asl + 1) * L],
                            ytile[:],
                            AF.Relu,
                            bias=xtile[:, u : u + 1],
                        )
                        asl += 1
                        n_act_left -= 1
                        if asl == G or n_act_left == 0:
                            pe_flush(wa, asl, sgnc)
                            wa = widepool.tile([128, G * L], BF16, tag="wa")
                            asl = 0
                    else:
                        nc.vector.tensor_scalar(
                            wd[:, ds * L : (ds + 1) * L],
                            ytile[:],
                            xtile[:, u : u + 1],
                            0.0,
                            op0=ALU.add,
                            op1=ALU.max,
                        )
                        ds += 1
                        n_dve_left -= 1
                        if ds == G or n_dve_left == 0:
                            pe_flush(wd, ds, sgnc)
                            wd = widepool.tile([128, G * L], BF16, tag="wd")
                            ds = 0

            # ---------------- final scalar ----------------
            red_row = wpool.tile([1, L], F32)
            nc.scalar.activation(red_row[:], ps_red[:], AF.Copy)
            red_sum = wpool.tile([1, 1], F32)
            nc.vector.tensor_reduce(
                red_sum[:], red_row[:], axis=mybir.AxisListType.X, op=ALU.add
            )

            # b2 * (N*L) folded bias term
            s_b2s = wpool.tile([CG, 1], F32)
            nc.vector.tensor_scalar(
                s_b2s[:], s_b2[:], float(N * L), None, op0=ALU.mult
            )
            ps_out = psum.tile([1, 1], F32, tag="mm")
            nc.tensor.matmul(ps_out[:], s_b2s[:], s_Wo[:], start=True, stop=True)
            bterm = wpool.tile([1, 1], F32)
            nc.vector.tensor_scalar(
                bterm[:], ps_out[:], s_bo[:, 0:1], None, op0=ALU.add
            )

            s_out = wpool.tile([1, 1], F32)
            nc.vector.tensor_tensor(s_out[:], red_sum[:], bterm[:], op=ALU.add)
            nc.sync.dma_start(out_d[:, :], s_out[:])

    nc.compile()
    return nc


def _shard(inputs):
    adj = np.ascontiguousarray(inputs["adj_mats"], np.float32)
    nodes = np.ascontiguousarray(inputs["nodes"], np.float32)
    prot = np.ascontiguousarray(inputs["protein_sequences"], np.float32)
    W1 = np.asarray(inputs["W1"], np.float32)

    base = np.zeros((128, WCOLS), np.float32)

    def put(name, arr, rows=slice(0, 128), coff=0):
        c0, _ = _WB[name]
        arr = np.asarray(arr, np.float32)
        base[rows, c0 + coff : c0 + coff + arr.shape[1]] = arr

    put("eye", np.eye(N, dtype=np.float32))
    put("Wg", inputs["Wg"])
    put("WrT", np.ascontiguousarray(np.asarray(inputs["Wr"], np.float32).T))
    put("W1t", W1[:CG])
    put("W1b", W1[CG:])
    put("W2T", np.ascontiguousarray(np.asarray(inputs["W2"], np.float32).T))
    put("Wn", inputs["Wn"], rows=slice(0, 64))
    put("cols3", np.asarray(inputs["Wo"], np.float32).reshape(CG, 1), coff=0)
    put("cols3", np.asarray(inputs["br"], np.float32).reshape(CP, 1), coff=1)
    put("cols3", np.asarray(inputs["b2"], np.float32).reshape(CG, 1), coff=2)
    put("rowvec", np.asarray(inputs["bn"], np.float32).reshape(1, CG), coff=0)
    put("rowvec", np.asarray(inputs["bg"], np.float32).reshape(1, CG), coff=128)
    put("rowvec", np.asarray(inputs["b1"], np.float32).reshape(1, HID), coff=256)
    put("rowvec", np.asarray(inputs["bo"], np.float32).reshape(1, 1), coff=448)

    in_maps = []
    for b in range(B):
        blob = base.copy()
        c0, _ = _WB["adj"]
        blob[:, c0 : c0 + N] = adj[b]
        c0, _ = _WB["nodesT"]
        blob[0:64, c0 : c0 + N] = nodes[b].T
        in_maps.append(
            {
                "wblob": blob,
                "protT": np.ascontiguousarray(prot[b].T),
            }
        )
    return in_maps


def _ensure_ntff_hook():
    """This container's `antenv` stub lacks axon_hooks; synthesize it from
    trn_boot's ctypes NTFF hook so run_bass_kernel_spmd(trace=True) works."""
    import types

    try:
        from antenv.axon_hooks import get_axon_ntff_profile_hook  # noqa: F401

        return
    except ImportError:
        pass
    try:
        from trn_agent_boot.trn_boot import _ntff_profile_via_ctypes

        hook = _ntff_profile_via_ctypes("/opt/axon/libaxon_pjrt.so")
    except Exception:
        hook = None
    mod = types.ModuleType("antenv.axon_hooks")
    mod._hook = hook
    mod.get_axon_ntff_profile_hook = lambda: mod._hook
    mod.set_axon_ntff_profile_hook = lambda h: setattr(mod, "_hook", h)
    import antenv

    antenv.axon_hooks = mod
    sys.modules["antenv.axon_hooks"] = mod


def _run(inputs, trace=False):
    if "nc" not in _CACHE:
        _CACHE["nc"] = _build()
    nc = _CACHE["nc"]
    if trace:
        _ensure_ntff_hook()
    res = run_bass_kernel_spmd(
        nc, _shard(inputs), core_ids=list(range(NCORES)), trace=trace
    )
    out = np.zeros((B, 1), np.float32)
    for b in range(B):
        out[b, 0] = np.asarray(res.results[b]["out"]).reshape(-1)[0]
    return out, res


def kernel(**inputs) -> np.ndarray:
    out, _ = _run(inputs, trace=False)
    return out


# revision 19
# speedup vs baseline: 1.9759x; 1.0751x over previous
"""Trainium2 Bass kernel for nn_DecomposableAttentionModel.

Math: the reference's softmax is over a size-1 axis, so attn == 1 exactly and

    out[b] = S[b] @ (W2 @ Wo) + N*L*(b2 @ Wo) + bo
    S[b,h] = sum_{n,l} relu(X[b,n,h] + Y[b,l,h])
    X      = gnn(adj, nodes) @ W1[:CG]
    Y      = prot @ (Wr @ W1[CG:]) + (br @ W1[CG:]) + b1

The [B, N*L, CG+CP] concat tensor is never materialized. Since
w*relu(z) = sign(w)*relu(|w|*z), |w2o| is folded into X and Y at PSUM
evacuation (free per-partition scale on the ACT copy) and the PE reduce
uses exact sign(w2o) stationary vectors.

Sharding: data-parallel over batch B=8, one graph per NeuronCore; weights
replicated. No collectives; the host assembles the [8,1] output. All
weights/biases ship as one packed [128, WCOLS] blob -> a single DMA.

Per-core hot loop (192 units: 128 chunk1 n's + 64 packed chunk2 pairs):
every unit is one relu pass (DVE tensor_scalar 2x | ACT activation) of
Y'[128,512]+x'_u into a wide bf16 buffer; PE folds each slot into
ps_red[1,512] via sign(w2o)^T @ slice matmuls accumulated in PSUM.
"""

import sys

if "/opt/trn_rl_repo" not in sys.path:
    sys.path.insert(0, "/opt/trn_rl_repo")

import numpy as np

import concourse.bacc as bacc
import concourse.bass as bass
import concourse.mybir as mybir
import concourse.tile as tile
from concourse.bass_utils import run_bass_kernel_spmd

B, N, NODE_DIM = 8, 128, 64
L, RES_DIM = 512, 1024
CG, CP = 128, 128
HID = CG + CP // 2  # 192
NUM_GNN_STEPS = 3
NCORES = 8

F32 = mybir.dt.float32
BF16 = mybir.dt.bfloat16
F8 = mybir.dt.float8e4
AF = mybir.ActivationFunctionType
ALU = mybir.AluOpType

# fraction of fused-loop units on the scalar (ACT) engine; rest on DVE.
ACT_FRAC = 0.30
G = 4  # wide-buffer slots per flush
SCALE8 = 256.0  # keeps |w2o|-scaled relu values in fp8e4 range

# packed weight-blob column layout: name -> (col_offset, cols)
_WB = {}
_c = 0
for _name, _w in [
    ("adj", N), ("eye", N), ("Wg", CG), ("WrT", RES_DIM), ("W1t", HID),
    ("W1b", HID), ("W2T", HID), ("Wn", N), ("nodesT", N), ("cols3", 3),
    ("rowvec", 449),
]:
    _WB[_name] = (_c, _w)
    _c += _w
WCOLS = _c

_CACHE = {}


def _build():
    nc = bacc.Bacc(
        "TRN2",
        target_bir_lowering=False,
        debug=False,
        num_devices=NCORES,
    )

    wblob = nc.dram_tensor("wblob", [128, WCOLS], F32, kind="ExternalInput").ap()
    protT = nc.dram_tensor("protT", [RES_DIM, L], F32, kind="ExternalInput").ap()
    out_d = nc.dram_tensor("out", [1, 1], F32, kind="ExternalOutput").ap()

    DT = RES_DIM // 128  # 8 k-tiles over the protein feature dim

    with tile.TileContext(nc) as tc:
        with (
            tc.tile_pool(name="const", bufs=1) as cpool,
            tc.tile_pool(name="work", bufs=1) as wpool,
            tc.tile_pool(name="wide", bufs=2) as widepool,
            tc.tile_pool(name="psum", bufs=2, space="PSUM") as psum,
            tc.tile_pool(name="psumY", bufs=1, space="PSUM") as psumY,
        ):
            # ---------------- loads (3 DMAs total) ----------------
            wb = cpool.tile([128, WCOLS], F32, tag="wblob")
            nc.sync.dma_start(wb[:], wblob[:, :])

            s_protall = cpool.tile([128, DT * L], F32, tag="prot")
            pt3 = protT.rearrange("(d p) l -> p d l", p=128)
            half = DT // 2
            nc.sync.dma_start(
                s_protall[:, 0 : half * L].rearrange("p (d l) -> p d l", d=half),
                pt3[:, 0:half, :],
            )
            nc.sync.dma_start(
                s_protall[:, half * L : DT * L].rearrange(
                    "p (d l) -> p d l", d=half
                ),
                pt3[:, half:DT, :],
            )

            def wbs(name, rows=slice(0, 128), coff=0, cols=None):
                c0, cw = _WB[name]
                return wb[rows, c0 + coff : c0 + coff + (cols or cw)]

            s_adj = wbs("adj")
            s_eye = wbs("eye")
            s_Wg = wbs("Wg")
            s_WrT = wbs("WrT")
            s_W1t = wbs("W1t")
            s_W1b = wbs("W1b")
            s_W2T = wbs("W2T")
            s_Wn = wbs("Wn", rows=slice(0, 64))
            s_nodesT = wbs("nodesT", rows=slice(0, 64))
            s_Wo = wbs("cols3", coff=0, cols=1)
            s_br = wbs("cols3", coff=1, cols=1)
            s_b2 = wbs("cols3", coff=2, cols=1)
            s_bn = wbs("rowvec", rows=slice(0, 1), coff=0, cols=CG)
            s_bg = wbs("rowvec", rows=slice(0, 1), coff=128, cols=CG)
            s_b1 = wbs("rowvec", rows=slice(0, 1), coff=256, cols=HID)
            s_bo = wbs("rowvec", rows=slice(0, 1), coff=448, cols=1)

            ones1 = cpool.tile([1, 128], F32)
            nc.gpsimd.memset(ones1[:], 1.0)
            ones512 = cpool.tile([1, L], F32)
            nc.gpsimd.memset(ones512[:], 1.0)

            # ---------------- w2o = W2 @ Wo; |w2o| scales + sign columns ----
            ps_w2oa = psum.tile([128, 1], F32, tag="mm")
            nc.tensor.matmul(ps_w2oa[:], s_W2T[:, 0:128], s_Wo[:], start=True, stop=True)
            absw1 = wpool.tile([128, 1], F32)
            nc.scalar.activation(absw1[:], ps_w2oa[:], AF.Abs, scale=SCALE8)
            sgn1 = wpool.tile([128, 1], BF16)
            nc.scalar.activation(sgn1[:], ps_w2oa[:], AF.Sign)
            sgn1_8 = wpool.tile([128, 32], F8)
            nc.gpsimd.memset(sgn1_8[:], 0.0)
            nc.scalar.activation(sgn1_8[:, 0:1], ps_w2oa[:], AF.Sign)
            nc.scalar.activation(sgn1_8[:, 16:17], ps_w2oa[:], AF.Sign)

            s_W2T2r = wpool.tile([CG, 128], F32)
            nc.vector.tensor_copy(s_W2T2r[:, 0:64], s_W2T[:, 128:HID])
            nc.vector.tensor_copy(s_W2T2r[:, 64:128], s_W2T[:, 128:HID])
            ps_w2ob = psum.tile([128, 1], F32, tag="mm")
            nc.tensor.matmul(ps_w2ob[:], s_W2T2r[:], s_Wo[:], start=True, stop=True)
            absw2 = wpool.tile([128, 1], F32)
            nc.scalar.activation(absw2[:], ps_w2ob[:], AF.Abs, scale=SCALE8)
            sgn2 = wpool.tile([128, 1], BF16)
            nc.scalar.activation(sgn2[:], ps_w2ob[:], AF.Sign)
            sgn2_8 = wpool.tile([128, 32], F8)
            nc.gpsimd.memset(sgn2_8[:], 0.0)
            nc.scalar.activation(sgn2_8[:, 0:1], ps_w2ob[:], AF.Sign)
            nc.scalar.activation(sgn2_8[:, 16:17], ps_w2ob[:], AF.Sign)

            # ---------------- adjacency normalization ----------------
            # At = diag(norm) @ A @ diag(norm), norm = clip(deg,1)^-0.5
            deg = wpool.tile([N, 1], F32)
            nc.vector.tensor_reduce(
                deg[:], s_adj[:], axis=mybir.AxisListType.X, op=ALU.add
            )
            nc.vector.tensor_scalar(deg[:], deg[:], 1.0, None, op0=ALU.max)
            sq = wpool.tile([N, 1], F32)
            nc.scalar.activation(sq[:], deg[:], AF.Sqrt)
            norm = wpool.tile([N, 1], F32)
            nc.vector.reciprocal(norm[:], sq[:])

            rowscaled = wpool.tile([N, N], F32)  # norm_i * A_ij
            nc.vector.tensor_scalar(
                rowscaled[:], s_adj[:], norm[:, 0:1], None, op0=ALU.mult
            )
            ps_rsT = psum.tile([N, N], F32, tag="mm")
            nc.tensor.transpose(ps_rsT[:], rowscaled[:], s_eye[:])
            s_At = wpool.tile([N, N], BF16)  # symmetric normalized adjacency
            nc.vector.tensor_scalar(
                s_At[:], ps_rsT[:], norm[:, 0:1], None, op0=ALU.mult
            )
            s_WgBF = wpool.tile([CG, CG], BF16)
            nc.vector.tensor_copy(s_WgBF[:], s_Wg[:])

            # ---------------- GNN ----------------
            # h0 = nodes @ Wn + bn   [N, CG]
            ps_h = psum.tile([N, CG], F32, tag="mm")
            nc.tensor.matmul(ps_h[:], s_nodesT[:], s_Wn[:], start=True, stop=False)
            nc.tensor.matmul(ps_h[:], ones1[:], s_bn[:], start=False, stop=True)
            s_h = wpool.tile([N, CG], BF16, tag="h")
            nc.scalar.activation(s_h[:], ps_h[:], AF.Copy)

            s_h3 = None
            for step in range(NUM_GNN_STEPS):
                last = step == NUM_GNN_STEPS - 1
                # uT = (At @ h)^T = h^T @ At   (At symmetric)  [CG, N]
                ps_uT = psum.tile([CG, N], F32, tag="uT")
                nc.tensor.matmul(ps_uT[:], s_h[:], s_At[:], start=True, stop=True)
                s_uT = wpool.tile([CG, N], BF16, tag="uT_s")
                nc.scalar.activation(s_uT[:], ps_uT[:], AF.Copy)
                # h' = act(u @ Wg + bg)   [N, CG]
                ps_h2 = psum.tile([N, CG], F32, tag="mm")
                nc.tensor.matmul(ps_h2[:], s_uT[:], s_WgBF[:], start=True, stop=False)
                nc.tensor.matmul(ps_h2[:], ones1[:], s_bg[:], start=False, stop=True)
                if last:
                    s_h3 = wpool.tile([N, CG], F32, tag="h3")
                    nc.scalar.activation(s_h3[:], ps_h2[:], AF.Tanh)
                else:
                    s_h = wpool.tile([N, CG], BF16, tag="h")
                    nc.scalar.activation(s_h[:], ps_h2[:], AF.Relu)

            # ---------------- X^T = W1t^T @ h3^T, scaled by |w2o| ----------
            ps_h3T = psum.tile([CG, N], F32, tag="uT")
            nc.tensor.transpose(ps_h3T[:], s_h3[:], s_eye[:])
            s_h3T = wpool.tile([CG, N], F32, tag="uT_s")
            nc.scalar.activation(s_h3T[:], ps_h3T[:], AF.Copy)

            ps_XT1 = psum.tile([128, N], F32, tag="mm")
            nc.tensor.matmul(ps_XT1[:], s_W1t[:, 0:128], s_h3T[:], start=True, stop=True)
            s_XT1 = wpool.tile([128, N], F32)
            nc.scalar.activation(s_XT1[:], ps_XT1[:], AF.Copy, scale=absw1[:, 0:1])

            # chunk-2 biases packed two-per-instruction directly out of PE:
            #   XP2[p<64, j] = X^T[128+p, j]; XP2[p>=64, j] = X^T[128+p-64, 64+j]
            w1t2a = wpool.tile([CG, 128], F32)
            w1t2b = wpool.tile([CG, 128], F32)
            nc.gpsimd.memset(w1t2a[:], 0.0)
            nc.gpsimd.memset(w1t2b[:], 0.0)
            nc.vector.tensor_copy(w1t2a[:, 0:64], s_W1t[:, 128:HID])
            nc.vector.tensor_copy(w1t2b[:, 64:128], s_W1t[:, 128:HID])
            ps_XP2 = psum.tile([128, N // 2], F32, tag="mm")
            nc.tensor.matmul(ps_XP2[:], w1t2a[:], s_h3T[:, 0:64], start=True, stop=False)
            nc.tensor.matmul(
                ps_XP2[:], w1t2b[:], s_h3T[:, 64:128], start=False, stop=True
            )
            s_XP2 = wpool.tile([128, N // 2], F32)
            nc.scalar.activation(s_XP2[:], ps_XP2[:], AF.Copy, scale=absw2[:, 0:1])

            # ---------------- Wc = Wr @ W1b  (-> bf16) ----------------
            s_Wc = []
            for d in range(DT):
                ps_wc = psum.tile([128, HID], F32, tag="mm")
                nc.tensor.matmul(
                    ps_wc[:],
                    s_WrT[:, d * 128 : (d + 1) * 128],
                    s_W1b[:],
                    start=True,
                    stop=True,
                )
                t = wpool.tile([128, HID], BF16, tag=f"wc{d}")
                nc.scalar.activation(t[:], ps_wc[:], AF.Copy)
                s_Wc.append(t)

            # c0 = br @ W1b + b1  (row [1, HID]) folded into Y
            ps_c0 = psum.tile([1, HID], F32, tag="mm")
            nc.tensor.matmul(ps_c0[:], s_br[:], s_W1b[:], start=True, stop=True)
            s_c0 = wpool.tile([1, HID], F32)
            nc.vector.tensor_tensor(s_c0[:], ps_c0[:], s_b1[:], op=ALU.add)

            # chunk-2 stationary tiles with duplicated columns, so the Y2
            # matmul lands already replicated across both partition halves
            s_Wc2r = []
            for d in range(DT):
                t = wpool.tile([128, 128], BF16, tag=f"wc2r{d}")
                nc.vector.tensor_copy(t[:, 0:64], s_Wc[d][:, 128:HID])
                nc.vector.tensor_copy(t[:, 64:128], s_Wc[d][:, 128:HID])
                s_Wc2r.append(t)
            s_c0rep = wpool.tile([1, 128], F32)
            nc.vector.tensor_copy(s_c0rep[:, 0:64], s_c0[:, 128:HID])
            nc.vector.tensor_copy(s_c0rep[:, 64:128], s_c0[:, 128:HID])

            # ---------------- protT -> bf16 ----------------
            s_pbfall = cpool.tile([128, DT * L], BF16, tag="pbf")
            for d in range(DT):
                nc.vector.tensor_copy(
                    s_pbfall[:, d * L : (d + 1) * L],
                    s_protall[:, d * L : (d + 1) * L],
                )

            def pbf(d):
                return s_pbfall[:, d * L : (d + 1) * L]

            # ------- Y^T = Wc^T @ protT + c0, scaled by |w2o|  [HID, L] -----
            ps_Y1 = psumY.tile([128, L], F32, tag="y1")
            ps_Y2 = psumY.tile([128, L], F32, tag="y2")
            for d in range(DT):
                nc.tensor.matmul(
                    ps_Y1[:], s_Wc[d][:, 0:128], pbf(d), start=(d == 0), stop=False
                )
            nc.tensor.matmul(
                ps_Y1[:], s_c0[:, 0:128], ones512[:], start=False, stop=True
            )
            for d in range(DT):
                nc.tensor.matmul(
                    ps_Y2[:], s_Wc2r[d][:], pbf(d), start=(d == 0), stop=False
                )
            nc.tensor.matmul(
                ps_Y2[:], s_c0rep[:], ones512[:], start=False, stop=True
            )

            s_Y1 = wpool.tile([128, L], BF16)
            nc.scalar.activation(s_Y1[:], ps_Y1[:], AF.Copy, scale=absw1[:, 0:1])
            s_Y2r = wpool.tile([128, L], BF16)
            nc.scalar.activation(s_Y2r[:], ps_Y2[:], AF.Copy, scale=absw2[:, 0:1])

            # ---------------- fused relu-sum loop ----------------
            # DVE units: bf16 relu pass (2x) -> wd slot; PE folds each slot as
            #   ps_red[1,512] += sgn_bf16^T @ slot          (512 PE cycles)
            # ACT units: fp8 relu pass -> wa slot; PE folds PAIRS of slots as
            #   ps_red += DoubleRow(sgn_fp8[128,2], [K,2,512]) (256 PE cycles)
            def spread(total, frac):
                k = int(round(total * frac))
                picks = set()
                for i in range(k):
                    picks.add(int(i * total / k))
                return [u in picks for u in range(total)]

            ps_red = psumY.tile([16, L], F32, tag="red")
            nc.vector.memset(ps_red[:], 0.0)
            chunks = [
                (s_Y1, s_XT1, sgn1, sgn1_8, N),
                (s_Y2r, s_XP2, sgn2, sgn2_8, N // 2),
            ]
            assigns = [spread(nu, ACT_FRAC) for (*_, nu) in chunks]

            # count PE-reduce matmuls: 1 per DVE unit; ceil(k/2) per ACT flush
            # of k slots (flushes happen at G slots or stream end per chunk)
            total_mms = 0
            for (*_, nunits), on_act in zip(chunks, assigns):
                nact = sum(on_act)
                ndve = nunits - nact
                total_mms += ndve
                full, rem = divmod(nact, G)
                total_mms += full * ((G + 1) // 2)
                if rem:
                    total_mms += (rem + 1) // 2
            mm_idx = [0]

            def red_mm(out_ap, *args, **kw):
                nc.tensor.matmul(
                    out_ap,
                    *args,
                    start=(mm_idx[0] == 0),
                    stop=(mm_idx[0] == total_mms - 1),
                    skip_group_check=True,
                    **kw,
                )
                mm_idx[0] += 1

            def flush_dve(widetile, nslots, sgnc):
                for s in range(nslots):
                    red_mm(ps_red[0:1, :], sgnc[:], widetile[:, s * L : (s + 1) * L])

            def flush_act(widetile, nslots, sgnc8):
                s = 0
                while s + 2 <= nslots:
                    rhs = widetile[:, s * L : (s + 2) * L].rearrange(
                        "k (r f) -> k r f", r=2
                    )
                    lhs = sgnc8[:].rearrange("k (r m) -> k r m", r=2)
                    red_mm(
                        ps_red[0:16, :],
                        lhs,
                        rhs,
                        perf_mode=mybir.MatmulPerfMode.DoubleRow,
                    )
                    s += 2
                if s < nslots:
                    red_mm(
                        ps_red[0:1, :],
                        sgnc8[:, 0:1],
                        widetile[:, s * L : (s + 1) * L],
                    )

            for (ytile, xtile, sgnc, sgnc8, nunits), on_act in zip(chunks, assigns):
                n_act_left = sum(on_act)
                n_dve_left = nunits - n_act_left
                wd = widepool.tile([128, G * L], BF16, tag="wd")
                wa = widepool.tile([128, G * L], F8, tag="wa")
                ds = asl = 0
                for u in range(nunits):
                    if on_act[u]:
                        nc.scalar.activation(
                            wa[:, asl * L : (asl + 1) * L],
                            ytile[:],
                            AF.Relu,
                            bias=xtile[:, u : u + 1],
                        )
                        asl += 1
                        n_act_left -= 1
                        if asl == G or n_act_left == 0:
                            flush_act(wa, asl, sgnc8)
                            wa = widepool.tile([128, G * L], F8, tag="wa")
                            asl = 0
                    else:
                        nc.vector.tensor_scalar(
                            wd[:, ds * L : (ds + 1) * L],
                            ytile[:],
                            xtile[:, u : u + 1],
                            0.0,
                            op0=ALU.add,
                            op1=ALU.max,
                        )
                        ds += 1
                        n_dve_left -= 1
                        if ds == G or n_dve_left == 0:
                            flush_dve(wd, ds, sgnc)
                            wd = widepool.tile([128, G * L], BF16, tag="wd")
                            ds = 0
            assert mm_idx[0] == total_mms, (mm_idx[0], total_mms)

            # ---------------- final scalar ----------------
            red_row = wpool.tile([1, L], F32)
            nc.scalar.activation(red_row[:], ps_red[0:1, :], AF.Copy)
            red_sum = wpool.tile([1, 1], F32)
            nc.vector.tensor_reduce(
                red_sum[:], red_row[:], axis=mybir.AxisListType.X, op=ALU.add
            )

            # b2 * (N*L) folded bias term
            s_b2s = wpool.tile([CG, 1], F32)
            nc.vector.tensor_scalar(
                s_b2s[:], s_b2[:], float(N * L), None, op0=ALU.mult
            )
            ps_out = psum.tile([1, 1], F32, tag="mm")
            nc.tensor.matmul(ps_out[:], s_b2s[:], s_Wo[:], start=True, stop=True)
            bterm = wpool.tile([1, 1], F32)
            nc.vector.tensor_scalar(
                bterm[:], ps_out[:], s_bo[:, 0:1], None, op0=ALU.add
            )

            red_sc = wpool.tile([1, 1], F32)
            nc.vector.tensor_scalar(
                red_sc[:], red_sum[:], 1.0 / SCALE8, None, op0=ALU.mult
            )
            s_out = wpool.tile([1, 1], F32)
            nc.vector.tensor_tensor(s_out[:], red_sc[:], bterm[:], op=ALU.add)
            nc.sync.dma_start(out_d[:, :], s_out[:])

    nc.compile()
    return nc


def _shard(inputs):
    adj = np.ascontiguousarray(inputs["adj_mats"], np.float32)
    nodes = np.ascontiguousarray(inputs["nodes"], np.float32)
    prot = np.ascontiguousarray(inputs["protein_sequences"], np.float32)
    W1 = np.asarray(inputs["W1"], np.float32)

    base = np.zeros((128, WCOLS), np.float32)

    def put(name, arr, rows=slice(0, 128), coff=0):
        c0, _ = _WB[name]
        arr = np.asarray(arr, np.float32)
        base[rows, c0 + coff : c0 + coff + arr.shape[1]] = arr

    put("eye", np.eye(N, dtype=np.float32))
    put("Wg", inputs["Wg"])
    put("WrT", np.ascontiguousarray(np.asarray(inputs["Wr"], np.float32).T))
    put("W1t", W1[:CG])
    put("W1b", W1[CG:])
    put("W2T", np.ascontiguousarray(np.asarray(inputs["W2"], np.float32).T))
    put("Wn", inputs["Wn"], rows=slice(0, 64))
    put("cols3", np.asarray(inputs["Wo"], np.float32).reshape(CG, 1), coff=0)
    put("cols3", np.asarray(inputs["br"], np.float32).reshape(CP, 1), coff=1)
    put("cols3", np.asarray(inputs["b2"], np.float32).reshape(CG, 1), coff=2)
    put("rowvec", np.asarray(inputs["bn"], np.float32).reshape(1, CG), coff=0)
    put("rowvec", np.asarray(inputs["bg"], np.float32).reshape(1, CG), coff=128)
    put("rowvec", np.asarray(inputs["b1"], np.float32).reshape(1, HID), coff=256)
    put("rowvec", np.asarray(inputs["bo"], np.float32).reshape(1, 1), coff=448)

    in_maps = []
    for b in range(B):
        blob = base.copy()
        c0, _ = _WB["adj"]
        blob[:, c0 : c0 + N] = adj[b]
        c0, _ = _WB["nodesT"]
        blob[0:64, c0 : c0 + N] = nodes[b].T
        in_maps.append(
            {
                "wblob": blob,
                "protT": np.ascontiguousarray(prot[b].T),
            }
        )
    return in_maps


def _ensure_ntff_hook():
    """This container's `antenv` stub lacks axon_hooks; synthesize it from
    trn_boot's ctypes NTFF hook so run_bass_kernel_spmd(trace=True) works."""
    import types

    try:
        from antenv.axon_hooks import get_axon_ntff_profile_hook  # noqa: F401

        return
    except ImportError:
        pass
    try:
        from trn_agent_boot.trn_boot import _ntff_profile_via_ctypes

        hook = _ntff_profile_via_ctypes("/opt/axon/libaxon_pjrt.so")
    except Exception:
        hook = None
    mod = types.ModuleType("antenv.axon_hooks")
    mod._hook = hook
    mod.get_axon_ntff_profile_hook = lambda: mod._hook
    mod.set_axon_ntff_profile_hook = lambda h: setattr(mod, "_hook", h)
    import antenv

    antenv.axon_hooks = mod
    sys.modules["antenv.axon_hooks"] = mod


def _run(inputs, trace=False):
    if "nc" not in _CACHE:
        _CACHE["nc"] = _build()
    nc = _CACHE["nc"]
    if trace:
        _ensure_ntff_hook()
    res = run_bass_kernel_spmd(
        nc, _shard(inputs), core_ids=list(range(NCORES)), trace=trace
    )
    out = np.zeros((B, 1), np.float32)
    for b in range(B):
        out[b, 0] = np.asarray(res.results[b]["out"]).reshape(-1)[0]
    return out, res


def kernel(**inputs) -> np.ndarray:
    out, _ = _run(inputs, trace=False)
    return out
